# revision 1
# baseline (speedup 1.0000x reference)
"""Trainium2 Bass kernel for nn_DiffAttention (GNN message passing), v2.

Math (per edge i: src s_i -> dst n, dst sorted):
  d_i = (h_dst[n] - h_src[s_i]) @ W_fc.T ;  e_i = tanh(d_i @ w_attn)
  alpha = segment_softmax(e, dst);  out[n] = elu(sum_i alpha_i d_i)
Since e in [-1,1], softmax needs no max-subtraction:
  out[n] = elu(p_dst[n] - (sum_i w_i p_src[s_i]) / (sum_i w_i)),
  w_i = exp(tanh(q_dst[n] - q_src[s_i])), p = h @ W_fc.T, q = p @ w_attn.

v2 design (8 cores, SPMD, edge-parallel by dst range):
  - bf16 node table [NPAD, 128]: [p_src(0:64) | q_src(64) | 1(65) | 0...],
    built sharded on PE from host-pre-transposed h_src tiles, AllGathered.
  - per-edge src rows fetched with dma_gather (InstDMAGatherAnt): int16
    indices limit addressing to <32768 rows, so the table is split in 4
    chunks of 25088 rows and each window's edges are bucketed by chunk
    on the host (bucket cap 512 edges = 4 tiles; windows shrink to fit).
    Two windows share one 1024-idx gather per chunk (4 gathers / 2 windows).
  - per window (<=128 consecutive dst nodes): host ships h_dst rows
    pre-transposed [64,128]; PE projects p_dst/q_dst on the fly.
    Per 128-edge tile: S01 one-hot (DVE bf16), qd = rowsum(S01*qb),
    batched tanh/exp on ACT, Sw = S01*w, PSUM acc += Sw.T @ pay[:,0:66].
  - epilogue: out = elu(p_dst - swp/sw) with zero-edge masking.
Host does only index prep / layout (windowing, bucketing, transposes).
"""
import sys
sys.path.insert(0, "/opt/trn_rl_repo")
import numpy as np
import ml_dtypes

N = 100000
D = 64
NC = 8
NCHUNK = 4
CHUNK = 25088          # table rows per gather chunk (int16-addressable)
NPAD = NCHUNK * CHUNK  # 100352
SHARD = NPAD // NC     # 12544 rows built per core
DUMMY = N              # zero row
CAPB = 512             # max edges per (window, chunk) bucket -> 4 tiles
KT = 16                # tiles per window (4 chunks x 4)
ROWE = 128             # bf16 elements per table row (256 B)
QCOL = 64              # q_src column
ONECOL = 65            # constant-1 column
MAIN_REPEAT = 1        # test.py overrides for timing

bf16 = ml_dtypes.bfloat16


# ---------------------------------------------------------------- host prep
def _partition_edges(dst):
    E = dst.shape[0]
    bounds, e_prev, n_prev = [], 0, 0
    for c in range(1, NC):
        s = (E * c) // NC
        while 0 < s < E and dst[s] == dst[s - 1]:
            s += 1
        node_split = int(dst[s]) if s < E else N
        bounds.append((e_prev, s, n_prev, node_split))
        e_prev, n_prev = s, node_split
    bounds.append((e_prev, E, n_prev, N))
    return bounds


def _build_windows(src, dst, e_lo, e_hi, n_lo, n_hi):
    """Split [n_lo, n_hi) into windows of <=128 nodes whose per-chunk edge
    buckets each hold <=CAPB edges. Returns per-window edge slices."""
    n_total = n_hi - n_lo
    counts = np.bincount(dst[e_lo:e_hi] - n_lo, minlength=n_total)
    starts = np.concatenate([[0], np.cumsum(counts)]) + e_lo
    chunk_of = (src[e_lo:e_hi] // CHUNK).astype(np.int8)
    # per-node, per-chunk counts for the greedy packer
    pc = np.zeros((n_total, NCHUNK), np.int32)
    node_rel = dst[e_lo:e_hi] - n_lo
    np.add.at(pc, (node_rel, chunk_of), 1)
    assert pc.max() <= CAPB, f"single node chunk-degree {pc.max()} > {CAPB}"
    wins = []  # (node_base_abs, n_nodes, e_start_abs, e_end_abs)
    n = 0
    while n < n_total:
        acc = np.zeros(NCHUNK, np.int64)
        n_end = n
        while n_end < n_total and n_end - n < 128:
            if np.any(acc + pc[n_end] > CAPB):
                break
            acc += pc[n_end]
            n_end += 1
        assert n_end > n
        wins.append((n_lo + n, n_end - n, int(starts[n]), int(starts[n_end])))
        n = n_end
    return wins


def _pack_idx(ix):
    """[1024] int16 -> [128, 64]: index i at [i%16, i//16], tiled x8."""
    return np.tile(ix.reshape(64, 16).T, (8, 1))


def _prep(src, dst, h_dst):
    src = np.asarray(src, np.int64)
    dst = np.asarray(dst, np.int64)
    if np.any(np.diff(dst) < 0):
        order = np.argsort(dst, kind="stable")
        src, dst = src[order], dst[order]
    bounds = _partition_edges(dst)
    per_core_wins = [_build_windows(src, dst, *b) for b in bounds]
    nW = max(len(w) for w in per_core_wins)
    if nW % 2:
        nW += 1
    nP = nW // 2
    chunk_all = (src // CHUNK).astype(np.int64)
    rel_all = (src - chunk_all * CHUNK).astype(np.int16)

    cores = []
    for wins in per_core_wins:
        idx_p = np.zeros((nP, 128, NCHUNK * 64), np.int16)
        dloc_p = np.full((nP, 128, 2 * KT), -1.0, np.float32)
        hdwT = np.zeros((nW, 64, 128), np.float32)
        base = np.full(nW, N, np.int64)
        nn = np.zeros(nW, np.int64)
        for p in range(nP):
            blocks = [np.zeros(1024, np.int16) for _ in range(NCHUNK)]
            for half in range(2):
                w = 2 * p + half
                if w >= len(wins):
                    continue
                b, k, es, ee = wins[w]
                base[w], nn[w] = b, k
                hdwT[w, :, :k] = h_dst[b:b + k].T
                ch = chunk_all[es:ee]
                rel = rel_all[es:ee]
                dl = (dst[es:ee] - b).astype(np.float32)
                order = np.argsort(ch, kind="stable")
                ch, rel, dl = ch[order], rel[order], dl[order]
                cnt = np.bincount(ch, minlength=NCHUNK)
                off = 0
                for c in range(NCHUNK):
                    m = int(cnt[c])
                    j = np.arange(m)
                    blocks[c][half * CAPB + j] = rel[off:off + m]
                    t = c * 4 + j // 128
                    dloc_p[p, j % 128, half * KT + t] = dl[off:off + m]
                    off += m
            for c in range(NCHUNK):
                idx_p[p, :, c * 64:(c + 1) * 64] = _pack_idx(blocks[c])
        cores.append(dict(idx=idx_p, dloc=dloc_p, hdwT=hdwT, base=base, nn=nn))
    return cores, nW


# ---------------------------------------------------------------- device
PAY_BUFS = 4


def _build_program(nW, main_repeat, ablate=""):
    from concourse import bass, bacc, mybir, tile
    f32, i16 = mybir.dt.float32, mybir.dt.int16
    bf = mybir.dt.bfloat16
    nP = nW // 2

    nc = bacc.Bacc("TRN2", target_bir_lowering=False, debug=False,
                   num_devices=NC)
    hsT_e = nc.dram_tensor("hsT", [SHARD // 128, 64, 128], f32,
                           kind="ExternalInput")
    wfc_e = nc.dram_tensor("wfc", [D, D], f32, kind="ExternalInput")
    wat_e = nc.dram_tensor("wat", [D, 1], f32, kind="ExternalInput")
    idx_e = nc.dram_tensor("idx", [nP, 128, NCHUNK * 64], i16,
                           kind="ExternalInput")
    dloc_e = nc.dram_tensor("dloc", [nP, 128, 2 * KT], f32,
                            kind="ExternalInput")
    hdwT_e = nc.dram_tensor("hdwT", [nW, 64, 128], f32, kind="ExternalInput")
    res_e = nc.dram_tensor("res", [nW * 128, D], f32, kind="ExternalOutput")

    with tile.TileContext(nc) as tc:
        with tc.tile_pool(name="c", bufs=1) as cp, \
             tc.tile_pool(name="sb", bufs=3) as sp, \
             tc.tile_pool(name="dr", bufs=1, space="DRAM") as dp:
            pp = tc.alloc_tile_pool(name="psb", bufs=1, space="PSUM")
            ident_d = nc.inline_tensor(np.eye(64, dtype=np.float32),
                                       name="ident_c")
            iota_d = nc.inline_tensor(
                np.tile(np.arange(128, dtype=np.float32),
                        (128, 1)).astype(bf16), name="iota_c")
            ident = cp.tile([64, 64], f32)
            nc.sync.dma_start(out=ident[:], in_=ident_d[:])
            iotab = cp.tile([128, 128], bf)
            nc.sync.dma_start(out=iotab[:], in_=iota_d[:])
            ones_row = cp.tile([1, 128], bf)
            nc.vector.memset(ones_row[:], 1.0)
            ones_col = cp.tile([128, 1], f32)
            nc.vector.memset(ones_col[:], 1.0)

            # rhsb [64, 66] = [W.T | v | 0], v = W.T @ w_attn
            wfc = cp.tile([D, D], f32)
            nc.sync.dma_start(out=wfc[:], in_=wfc_e[:])
            wat = cp.tile([D, 1], f32)
            nc.sync.dma_start(out=wat[:], in_=wat_e[:])
            wt_ps = pp.tile([D, D], f32, space="PSUM")
            nc.tensor.transpose(out=wt_ps[:], in_=wfc[:], identity=ident[:])
            v_ps = pp.tile([D, 1], f32, space="PSUM")
            nc.tensor.matmul(out=v_ps[:], lhsT=wfc[:], rhs=wat[:],
                             start=True, stop=True)
            rhsb = cp.tile([D, 66], f32)
            nc.vector.memset(rhsb[:], 0.0)
            nc.vector.tensor_copy(rhsb[:, 0:64], wt_ps[:])
            nc.vector.tensor_copy(rhsb[:, 64:65], v_ps[:])
            vcol = cp.tile([D, 1], f32)
            nc.vector.tensor_copy(vcol[:], v_ps[:])

            # ---- src table build (this core's shard)
            tbl_sh = dp.tile([SHARD, ROWE], bf)
            for j in range(SHARD // 128):
                hsT = sp.tile([64, 128], f32, tag="bh")
                nc.sync.dma_start(out=hsT[:], in_=hsT_e[j])
                pb = pp.tile([128, 66], f32, space="PSUM", tag="bp")
                nc.tensor.matmul(out=pb[:], lhsT=hsT[:], rhs=rhsb[:],
                                 start=True, stop=True)
                tb = sp.tile([128, ROWE], bf, tag="bo")
                nc.vector.memset(tb[:], 0.0)
                nc.vector.tensor_copy(tb[:, 0:65], pb[:, 0:65])
                nc.vector.memset(tb[:, ONECOL:ONECOL + 1], 1.0)
                nc.sync.dma_start(out=tbl_sh[j * 128:(j + 1) * 128, :],
                                  in_=tb[:])

            pp.release()
            pp2 = tc.alloc_tile_pool(name="psm", bufs=2, space="PSUM")

            # ---- all-gather the table
            table = dp.tile([NPAD, ROWE], bf)
            nc.gpsimd.collective_compute(
                "AllGather", mybir.AluOpType.bypass,
                replica_groups=[list(range(NC))],
                ins=[tbl_sh.opt()], outs=[table.opt()])

            # ---- main loop over window pairs
            rep_ctx = tc.For_i(0, main_repeat, 1) if main_repeat > 1 else None
            if rep_ctx is not None:
                rep_ctx.__enter__()
            for p in range(nP):
                idxt = sp.tile([128, NCHUNK * 64], i16, tag="idx")
                nc.sync.dma_start(out=idxt[:], in_=idx_e[p])
                dloc = sp.tile([128, 2 * KT], f32, tag="dl")
                nc.sync.dma_start(out=dloc[:], in_=dloc_e[p])
                pay = sp.tile([128, NCHUNK * 8, ROWE], bf, tag="pay",
                              bufs=PAY_BUFS)
                if ablate == "compute_only":
                    nc.vector.memset(pay[:], 0.0)
                else:
                    for c in range(NCHUNK):
                        nc.gpsimd.dma_gather(
                            pay[:, c * 8:(c + 1) * 8, :],
                            table[c * CHUNK:(c + 1) * CHUNK, :],
                            idxt[:, c * 64:(c + 1) * 64],
                            1024, 1024, ROWE, queue_num=0)
                if ablate == "gather_only":
                    acc = pp2.tile([128, 66], f32, space="PSUM", tag="acc")
                    nc.tensor.matmul(out=acc[:], lhsT=iotab[:],
                                     rhs=pay[:, 0, 0:66], start=True,
                                     stop=True)
                    resg = sp.tile([128, D], f32, tag="res")
                    nc.vector.tensor_copy(resg[:], acc[:, 0:64])
                    nc.sync.dma_start(
                        out=res_e[2 * p * 128:(2 * p + 1) * 128, :],
                        in_=resg[:])
                    continue
                for half in range(2):
                    w = 2 * p + half
                    hdwT = sp.tile([64, 128], f32, tag="hw")
                    nc.sync.dma_start(out=hdwT[:], in_=hdwT_e[w])
                    pb = pp2.tile([128, 66], f32, space="PSUM", tag="pb")
                    nc.tensor.matmul(out=pb[:], lhsT=hdwT[:], rhs=rhsb[:],
                                     start=True, stop=True)
                    pbs = sp.tile([128, 66], f32, tag="pbs")
                    nc.vector.tensor_copy(pbs[:], pb[:])
                    qrow_ps = pp2.tile([1, 128], f32, space="PSUM", tag="qr")
                    nc.tensor.matmul(out=qrow_ps[:], lhsT=vcol[:],
                                     rhs=hdwT[:], start=True, stop=True)
                    qrow = sp.tile([1, 128], bf, tag="qrs")
                    nc.vector.tensor_copy(qrow[:], qrow_ps[:])
                    qb_ps = pp2.tile([128, 128], f32, space="PSUM", tag="qb")
                    nc.tensor.matmul(out=qb_ps[:], lhsT=ones_row[:],
                                     rhs=qrow[:], start=True, stop=True)
                    qb = sp.tile([128, 128], bf, tag="qbs")
                    nc.vector.tensor_copy(qb[:], qb_ps[:])

                    S01a = sp.tile([128, KT, 128], bf, tag="s01", bufs=2)
                    qd = sp.tile([128, KT], f32, tag="qd")
                    scr = sp.tile([128, 128], bf, tag="scr", bufs=4)
                    for t in range(KT):
                        pt = (t // 4) * 8 + half * 4 + (t % 4)
                        dcol = dloc[:, half * KT + t:half * KT + t + 1]
                        nc.vector.tensor_scalar(
                            out=S01a[:, t, :], in0=iotab[:], scalar1=dcol,
                            scalar2=None, op0=mybir.AluOpType.is_equal)
                        nc.vector.tensor_tensor(
                            scr[:], S01a[:, t, :], qb[:],
                            op=mybir.AluOpType.mult)
                        nc.vector.tensor_reduce(
                            out=qd[:, t:t + 1], in_=scr[:],
                            axis=mybir.AxisListType.X, op=mybir.AluOpType.add)
                    # dq = qd - q_src, per chunk (qs strided out of pay)
                    dq = sp.tile([128, KT], f32, tag="dq")
                    for c in range(NCHUNK):
                        t0 = c * 8 + half * 4
                        nc.vector.tensor_tensor(
                            dq[:, c * 4:(c + 1) * 4],
                            qd[:, c * 4:(c + 1) * 4],
                            pay[:, t0:t0 + 4, QCOL:QCOL + 1],
                            op=mybir.AluOpType.subtract)
                    th = sp.tile([128, KT], f32, tag="th")
                    nc.scalar.activation(
                        out=th[:], in_=dq[:],
                        func=mybir.ActivationFunctionType.Tanh)
                    wc = sp.tile([128, KT], f32, tag="wc")
                    nc.scalar.activation(
                        out=wc[:], in_=th[:],
                        func=mybir.ActivationFunctionType.Exp)

                    acc = pp2.tile([128, 66], f32, space="PSUM", tag="acc")
                    Sw = sp.tile([128, 128], bf, tag="sw", bufs=4)
                    for t in range(KT):
                        pt = (t // 4) * 8 + half * 4 + (t % 4)
                        nc.vector.tensor_scalar(
                            out=Sw[:], in0=S01a[:, t, :],
                            scalar1=wc[:, t:t + 1], scalar2=None,
                            op0=mybir.AluOpType.mult)
                        nc.tensor.matmul(out=acc[:], lhsT=Sw[:],
                                         rhs=pay[:, pt, 0:66],
                                         start=(t == 0), stop=(t == KT - 1))

                    # epilogue: out = elu(p_dst - swp/sw) * (sw != 0)
                    z = sp.tile([128, 1], f32, tag="z")
                    nc.vector.tensor_scalar(
                        out=z[:], in0=acc[:, ONECOL:ONECOL + 1], scalar1=0.0,
                        scalar2=None, op0=mybir.AluOpType.is_equal)
                    den = sp.tile([128, 1], f32, tag="den")
                    nc.vector.tensor_tensor(den[:], acc[:, ONECOL:ONECOL + 1],
                                            z[:], op=mybir.AluOpType.add)
                    rec = sp.tile([128, 1], f32, tag="rec")
                    nc.vector.reciprocal(rec[:], den[:])
                    nzm = sp.tile([128, 1], f32, tag="nzm")
                    nc.vector.scalar_tensor_tensor(
                        out=nzm[:], in0=z[:], scalar=-1.0, in1=ones_col[:],
                        op0=mybir.AluOpType.mult, op1=mybir.AluOpType.add)
                    mean = sp.tile([128, D], f32, tag="mean")
                    nc.vector.tensor_scalar(
                        out=mean[:], in0=acc[:, 0:64], scalar1=rec[:],
                        scalar2=None, op0=mybir.AluOpType.mult)
                    diff = sp.tile([128, D], f32, tag="diff")
                    nc.vector.tensor_tensor(diff[:], pbs[:, 0:64], mean[:],
                                            op=mybir.AluOpType.subtract)
                    dm = sp.tile([128, D], f32, tag="dm")
                    nc.vector.tensor_scalar(
                        out=dm[:], in0=diff[:], scalar1=nzm[:], scalar2=None,
                        op0=mybir.AluOpType.mult)
                    neg = sp.tile([128, D], f32, tag="neg")
                    nc.vector.tensor_scalar(
                        out=neg[:], in0=dm[:], scalar1=0.0, scalar2=None,
                        op0=mybir.AluOpType.min)
                    ex = sp.tile([128, D], f32, tag="ex")
                    nc.scalar.activation(out=ex[:], in_=neg[:],
                                         func=mybir.ActivationFunctionType.Exp)
                    pos = sp.tile([128, D], f32, tag="pos")
                    nc.vector.tensor_scalar(
                        out=pos[:], in0=dm[:], scalar1=0.0, scalar2=None,
                        op0=mybir.AluOpType.max)
                    res = sp.tile([128, D], f32, tag="res")
                    nc.vector.scalar_tensor_tensor(
                        out=res[:], in0=ex[:], scalar=-1.0, in1=pos[:],
                        op0=mybir.AluOpType.add, op1=mybir.AluOpType.add)
                    nc.sync.dma_start(out=res_e[w * 128:(w + 1) * 128, :],
                                      in_=res[:])
            if rep_ctx is not None:
                rep_ctx.__exit__(None, None, None)
            pp2.release()
    nc.compile()
    return nc


_CACHE = {}


def _get_program(nW, main_repeat, ablate=""):
    key = (nW, main_repeat, ablate)
    if key not in _CACHE:
        _CACHE[key] = _build_program(nW, main_repeat, ablate)
    return _CACHE[key]


def kernel(h_src, h_dst, W_fc, w_attn, src, dst, _main_repeat=MAIN_REPEAT,
           _ablate=""):
    from concourse.bass_utils import run_bass_kernel_spmd

    h_src = np.ascontiguousarray(np.asarray(h_src, np.float32))
    h_dst = np.ascontiguousarray(np.asarray(h_dst, np.float32))
    W_fc = np.ascontiguousarray(np.asarray(W_fc, np.float32))
    w_attn = np.ascontiguousarray(np.asarray(w_attn, np.float32)).reshape(D, 1)
    cores, nW = _prep(src, dst, h_dst)

    hp = np.zeros((NPAD, D), np.float32)
    hp[:N] = h_src
    hsT_all = hp.reshape(NPAD // 128, 128, D).transpose(0, 2, 1).copy()

    in_maps = []
    for c, core in enumerate(cores):
        in_maps.append({
            "hsT": hsT_all[c * (SHARD // 128):(c + 1) * (SHARD // 128)],
            "wfc": W_fc,
            "wat": w_attn,
            "idx": core["idx"],
            "dloc": core["dloc"],
            "hdwT": core["hdwT"],
        })
    nc = _get_program(nW, _main_repeat, _ablate)
    res = run_bass_kernel_spmd(nc, in_maps, list(range(NC)))

    out = np.zeros((N, D), np.float32)
    for c, core in enumerate(cores):
        r = res.results[c]["res"].reshape(nW, 128, D)
        base, nn = core["base"], core["nn"]
        for w in range(nW):
            if nn[w] > 0:
                out[base[w]:base[w] + nn[w]] = r[w, :nn[w]]
    return out


if __name__ == "__main__":
    d = np.load("/root/problem/refdata.npz")
    out = kernel(d["h_src"], d["h_dst"], d["W_fc"], d["w_attn"],
                 d["src"], d["dst"])
    exp = d["expected"]
    rel = np.linalg.norm(out - exp) / np.linalg.norm(exp)
    print(f"rel_l2 = {rel:.3e}  maxabs = {np.abs(out - exp).max():.3e}")

